# revision 3
# baseline (speedup 1.0000x reference)
"""Trainium2 Bass kernel for a FlowNet-style CorrelationLayer.

out[0, j*7+i, h, w] = sum_c x[0,c,h,w] * y[0,c,h+j-3,w+i-3]   (zero-padded y)

Shapes: x, y = [1, 128, 384, 512] fp32  ->  out = [1, 49, 384, 512] fp32.

Strategy
--------
* Shard H (rows) across the 8 NeuronCores: core k computes output rows
  [48k, 48k+48).  The y halo (3 rows each side) is sliced on the host from
  the full input, so no inter-core communication is needed.
* Per core, the C=128 contraction runs on the TensorEngine as "all-pairs"
  patch matmuls: lhsT = an 8x8 pixel patch of x (M=64 columns, K=C=128),
  rhs = the matching 14x14 halo patch of y (N=196 columns).  Entry
  (m=(a,b), n=(u,v)) of the PSUM block is the correlation of x pixel
  (a,b) with y pixel (u-3, v-3) relative to the patch origin, so the 49
  shift planes live on 49 diagonals of each block.  Two patches are packed
  into the 128 PE columns via tile_position col-tiling.
* All blocks are evacuated (fp32 -> fp16) into one staging tensor laid out
  [128 part = pixel-in-patch, 196 = halo pos, 192 = patch index (pr*32+q)].
  With the halo position OUTER and the patch index INNER, the elements of
  one output plane (fixed shift, fixed pixel class) are contiguous runs of
  192, and runs for the 7 horizontal shifts i of one vertical shift j are
  adjacent -> 1344-element (2688 B) contiguous descriptors.
* The banded "diagonal extraction" therefore becomes 16 plain DMAs (one
  per (column-tile h, patch row a)): the b dimension is expressed as a
  mixed stride (+1 partition, +192 elements) in the flat SBUF address
  space, j is a strided dim, and (i, pr, q) is the contiguous run.  Only
  the needed 49 values/pixel (2.4 MB/core) ever leave the chip, vs 9.6 MB
  for a full-block dump; the final reassembly is a pure numpy transpose.
"""

import numpy as np

import concourse.bass as bass
import concourse.tile as tile
from concourse import bacc, mybir
from concourse.bass_utils import run_bass_kernel_spmd

B, C, H, W = 1, 128, 384, 512
NCORES = 8
HB = H // NCORES          # 48 output rows per core
PA, PB = 8, 8             # x patch: 8 rows x 8 cols = 64 = M per matmul
HA, HW_ = PA + 6, PB + 6  # y halo patch: 14 x 14
NF = HA * HW_             # 196 = N (matmul free size)
PR = HB // PA             # 6 patch-rows
PW = W // PB              # 64 patch-cols
PQ = PW // 2              # 32 pairs (two patches packed per 128 partitions)
NPQ = PR * PQ             # 192 = patch index within the core band
PITCH = NF * NPQ          # staging free size per partition (elements)

F16 = mybir.dt.float16

_PROGRAM = None


def _build_program():
    nc = bacc.Bacc("TRN2", target_bir_lowering=False, debug=False)

    xb = nc.declare_dram_parameter("xb", [C, PR * PW, PA * PB], F16, isOutput=False)
    yb = nc.declare_dram_parameter("yb", [C, HB + 6, W + 6], F16, isOutput=False)
    # corr[m, j, i*192 + pr*32 + q] = correlation of pixel class m=(h,a,b)
    # of patch (pr, q) with shift (j-3, i-3).
    corr = nc.declare_dram_parameter("corr", [128, 7, 7 * NPQ], F16, isOutput=True)

    with tile.TileContext(nc) as tc:
        with (
            tc.tile_pool(name="xpool", bufs=1) as xpool,
            tc.tile_pool(name="ypool", bufs=1) as ypool,
            tc.tile_pool(name="stpool", bufs=1) as stpool,
            tc.tile_pool(name="psum", bufs=4, space="PSUM") as psum_pool,
        ):
            X = xpool.tile([C, PR * PW, PA * PB], F16)
            Y = ypool.tile([C, HB + 6, W + 6], F16)
            ST = stpool.tile([128, NF, NPQ], F16)

            # Issue input loads in the order the patch-row pipeline consumes
            # them (the HW queue drains FIFO): patch-row pr needs X chunk pr
            # and Y rows [8pr, 8pr+14) = Y chunks pr and pr+1.
            def load_x(pr):
                nc.sync.dma_start(
                    X[:, pr * PW : (pr + 1) * PW, :], xb[:, pr * PW : (pr + 1) * PW, :]
                )

            def load_y(ch):  # Y chunk = 8 rows (last chunk 6 rows)
                r0, r1 = ch * 8, min(ch * 8 + 8, HB + 6)
                nc.sync.dma_start(Y[:, r0:r1, :], yb[:, r0:r1, :])

            load_x(0)
            load_y(0)
            load_y(1)
            for pr in range(1, PR):
                load_x(pr)
                load_y(pr + 1)

            for pr in range(PR):
                for qq in range(0, PQ, 2):
                    # Four 8x8 patches (two col-tiled pairs) share one PSUM
                    # bank; their evacuation is a single strided copy.
                    ps = psum_pool.tile([128, 2, 256], mybir.dt.float32)
                    for s in range(2):
                        q = qq + s
                        for half in range(2):
                            wp = 2 * q + half
                            lhsT = X[:, pr * PW + wp, :]
                            rhs = Y[
                                :, pr * PA : pr * PA + HA, wp * PB : wp * PB + HW_
                            ]
                            nc.tensor.matmul(
                                ps[half * 64 : (half + 1) * 64, s, :NF],
                                lhsT,
                                rhs,
                                start=True,
                                stop=True,
                                tile_position=(0, 64 * half),
                            )
                    # dst enumerates (q, n) to match the PSUM source order:
                    # element (q, n) lands at free offset n*192 + pr*32+qq+q.
                    dst = ST[:, :, pr * PQ + qq : pr * PQ + qq + 2].transpose(
                        [0, 2, 1]
                    )
                    # Alternate evacuation between DVE and ACT so neither
                    # becomes the bottleneck.
                    if (qq // 2) % 2 == 0:
                        nc.vector.tensor_copy(dst, ps[:, :, :NF])
                    else:
                        nc.scalar.copy(dst, ps[:, :, :NF])

            # Banded extraction: partition 64h+8a+b needs halo positions
            # n = (a+j)*14 + (b+i): per (a, b) that is one DMA whose dim0
            # covers the two col-tile halves (partition step 64), j is a
            # strided dim, and (i, pr, q) is a contiguous 1344-element run.
            # The BIR verifier forbids access-pattern dims that cross
            # partitions with a free-offset component, so a and b cannot be
            # folded in and 64 DMAs are required; their dispatch cost is
            # spread over the SP, ACT and Pool sequencers.
            ST_t = ST[:, :, :].tensor
            engs = [nc.sync, nc.scalar, nc.gpsimd]
            k = 0
            for a in range(PA):
                for b in range(PB):
                    m0 = 8 * a + b
                    src = bass.AP(
                        ST_t,
                        m0 * PITCH + (14 * a + b) * NPQ,
                        [(64 * PITCH, 2), (14 * NPQ, 7), (1, 7 * NPQ)],
                    )
                    dst = corr[m0::64, :, :]
                    engs[k % 3].dma_start(dst, src)
                    k += 1

    nc.compile()
    return nc


def _program():
    global _PROGRAM
    if _PROGRAM is None:
        _PROGRAM = _build_program()
    return _PROGRAM


def _make_in_maps(x: np.ndarray, y: np.ndarray):
    x0 = np.asarray(x[0]).astype(np.float16)
    # [C, H, W] -> [C, H/PA, PA, PW, PB] -> [C, H/PA, PW, PA, PB]
    xt = x0.reshape(C, H // PA, PA, PW, PB).transpose(0, 1, 3, 2, 4)
    xt = np.ascontiguousarray(xt.reshape(C, H // PA * PW, PA * PB))
    yp = np.zeros((C, H + 6, W + 6), np.float16)
    yp[:, 3 : 3 + H, 3 : 3 + W] = y[0]
    in_maps = []
    for k in range(NCORES):
        in_maps.append(
            {
                "xb": np.ascontiguousarray(xt[:, k * PR * PW : (k + 1) * PR * PW, :]),
                "yb": np.ascontiguousarray(yp[:, k * HB : k * HB + HB + 6, :]),
            }
        )
    return in_maps


def _gather_core(corr_k: np.ndarray) -> np.ndarray:
    """[128, 7, 7*192] -> [49, HB, W] band of the output."""
    r = corr_k.reshape(2, PA, PB, 7, 7, PR, PQ)  # [h, a, b, j, i, pr, q]
    # out[(j,i), pr*8+a, q*16 + h*8 + b]
    return r.transpose(3, 4, 5, 1, 6, 0, 2).reshape(49, HB, W)


def _run(in_maps, trace=False, **kw):
    return run_bass_kernel_spmd(
        _program(), in_maps, core_ids=list(range(NCORES)), trace=trace, **kw
    )


def kernel(x: np.ndarray, y: np.ndarray) -> np.ndarray:
    x = np.asarray(x)
    y = np.asarray(y)
    res = _run(_make_in_maps(x, y)).results
    out = np.empty((1, 49, H, W), np.float32)
    for k in range(NCORES):
        out[0, :, k * HB : (k + 1) * HB, :] = _gather_core(
            np.asarray(res[k]["corr"])
        ).astype(np.float32)
    return out


# revision 5
# speedup vs baseline: 1.7465x; 1.7465x over previous
"""Trainium2 Bass kernel for a FlowNet-style CorrelationLayer.

out[0, j*7+i, h, w] = sum_c x[0,c,h,w] * y[0,c,h+j-3,w+i-3]   (zero-padded y)

Shapes: x, y = [1, 128, 384, 512] fp32  ->  out = [1, 49, 384, 512] fp32.

Strategy
--------
* Shard H (rows) across the 8 NeuronCores: core k computes output rows
  [48k, 48k+48).  The y halo (3 rows each side) is sliced on the host from
  the full input, so no inter-core communication is needed.
* Per core, the C=128 contraction runs on the TensorEngine as "all-pairs"
  patch matmuls: lhsT = an 8x8 pixel patch of x (M=64 columns, K=C=128),
  rhs = the matching 14x14 halo patch of y (N=196 columns).  Entry
  (m=(a,b), n=(al,be)) of the PSUM block is the correlation of x pixel
  (a,b) with y pixel (al-3, be-3) relative to the patch origin, so the 49
  shift planes live on 49 diagonals of each block.  Two patches are packed
  into the 128 PE columns via tile_position col-tiling so partitions (and
  hence DMA width) stay full.
* Diagonal extraction is not expressible with uniform per-partition access
  patterns on any engine (the BIR verifier forbids access-pattern dims
  that combine a partition step with a free-offset component, and
  per-(a,b) banded DMAs serialize on the few DGE queues), so each PSUM
  block is cast to fp16 and dumped whole to DRAM; the final banded gather
  is a cheap numpy fancy-index on the host.  Inputs are also shipped as
  fp16 (quantization error ~4e-4 relative, well within tolerance).  Total
  HBM traffic per core is ~23 MB, close to the memory roofline.
* Evacuation of PSUM blocks alternates DVE / ACT / GpSimd so no single
  engine paces the late (input-starved) patch rows.
"""

import numpy as np

import concourse.bass as bass  # noqa: F401  (AP types pulled in transitively)
import concourse.tile as tile
from concourse import bacc, mybir
from concourse.bass_utils import run_bass_kernel_spmd

B, C, H, W = 1, 128, 384, 512
NCORES = 8
HB = H // NCORES          # 48 output rows per core
PA, PB = 8, 8             # x patch: 8 rows x 8 cols = 64 = M per matmul
HA, HW_ = PA + 6, PB + 6  # y halo patch: 14 x 14
NF = HA * HW_             # 196 = N (matmul free size)
PR = HB // PA             # 6 patch-rows
PW = W // PB              # 64 patch-cols
PQ = PW // 2              # 32 pairs (two patches packed per 128 partitions)

F16 = mybir.dt.float16

_PROGRAM = None


def _build_program():
    nc = bacc.Bacc("TRN2", target_bir_lowering=False, debug=False)

    # x is pre-tiled on the host to [C, patch, m] so each patch's 64 weight
    # columns are contiguous (walrus requires a single free dim on the
    # stationary matmul operand).
    xb = nc.declare_dram_parameter("xb", [C, PR * PW, PA * PB], F16, isOutput=False)
    yb = nc.declare_dram_parameter("yb", [C, HB + 6, W + 6], F16, isOutput=False)
    corr = nc.declare_dram_parameter("corr", [PR, 128, PQ, NF], F16, isOutput=True)

    with tile.TileContext(nc) as tc:
        with (
            tc.tile_pool(name="xpool", bufs=1) as xpool,
            tc.tile_pool(name="ypool", bufs=1) as ypool,
            tc.tile_pool(name="psum", bufs=4, space="PSUM") as psum_pool,
            tc.tile_pool(name="stage", bufs=2) as stage_pool,
        ):
            X = xpool.tile([C, PR * PW, PA * PB], F16)
            Y = ypool.tile([C, HB + 6, W + 6], F16)

            # Issue input loads in the order the patch-row pipeline consumes
            # them (the HW queue drains FIFO): patch-row pr needs X chunk pr
            # and Y rows [8pr, 8pr+14) = Y chunks pr and pr+1.
            def load_x(pr):
                nc.sync.dma_start(
                    X[:, pr * PW : (pr + 1) * PW, :], xb[:, pr * PW : (pr + 1) * PW, :]
                )

            def load_y(ch):  # Y chunk = 8 rows (last chunk 6 rows)
                r0, r1 = ch * 8, min(ch * 8 + 8, HB + 6)
                nc.sync.dma_start(Y[:, r0:r1, :], yb[:, r0:r1, :])

            load_x(0)
            load_y(0)
            load_y(1)
            for pr in range(1, PR):
                load_x(pr)
                load_y(pr + 1)

            for pr in range(PR):
                # One staging buffer and two output DMAs per patch-row keep
                # the Sync sequencer's per-DMA dispatch (~0.6us) off the
                # critical path.
                st = stage_pool.tile([128, PQ, NF], F16)
                for qq in range(0, PQ, 2):
                    # Four 8x8 patches (two col-tiled pairs) share one PSUM
                    # bank; their evacuation is a single strided copy.
                    ps = psum_pool.tile([128, 2, 256], mybir.dt.float32)
                    for s in range(2):
                        q = qq + s
                        for half in range(2):
                            wp = 2 * q + half
                            lhsT = X[:, pr * PW + wp, :]
                            rhs = Y[
                                :, pr * PA : pr * PA + HA, wp * PB : wp * PB + HW_
                            ]
                            nc.tensor.matmul(
                                ps[half * 64 : (half + 1) * 64, s, :NF],
                                lhsT,
                                rhs,
                                start=True,
                                stop=True,
                                tile_position=(0, 64 * half),
                            )
                    dst = st[:, qq : qq + 2, :]
                    # Alternate evacuation between DVE and ACT so neither
                    # becomes the bottleneck.
                    if (qq // 2) % 2 == 0:
                        nc.vector.tensor_copy(dst, ps[:, :, :NF])
                    else:
                        nc.scalar.copy(dst, ps[:, :, :NF])
                    if qq == PQ // 2 - 2:
                        # First half of the row band is done — ship it while
                        # the second half is still being computed.
                        nc.sync.dma_start(
                            corr[pr, :, : PQ // 2], st[:, : PQ // 2, :]
                        )
                nc.sync.dma_start(corr[pr, :, PQ // 2 :], st[:, PQ // 2 :, :])

    nc.compile()
    return nc


def _program():
    global _PROGRAM
    if _PROGRAM is None:
        _PROGRAM = _build_program()
    return _PROGRAM


def _make_in_maps(x: np.ndarray, y: np.ndarray):
    x0 = np.asarray(x[0]).astype(np.float16)
    # [C, H, W] -> [C, H/PA, PA, PW, PB] -> [C, H/PA, PW, PA, PB]
    xt = x0.reshape(C, H // PA, PA, PW, PB).transpose(0, 1, 3, 2, 4)
    xt = np.ascontiguousarray(xt.reshape(C, H // PA * PW, PA * PB))
    yp = np.zeros((C, H + 6, W + 6), np.float16)
    yp[:, 3 : 3 + H, 3 : 3 + W] = y[0]
    in_maps = []
    for k in range(NCORES):
        in_maps.append(
            {
                "xb": np.ascontiguousarray(xt[:, k * PR * PW : (k + 1) * PR * PW, :]),
                "yb": np.ascontiguousarray(yp[:, k * HB : k * HB + HB + 6, :]),
            }
        )
    return in_maps


_GATHER_IDX = None


def _gather_indices():
    global _GATHER_IDX
    if _GATHER_IDX is None:
        a = np.arange(PA)[:, None, None, None]
        b = np.arange(PB)[None, :, None, None]
        j = np.arange(7)[None, None, :, None]
        i = np.arange(7)[None, None, None, :]
        # n offset for pixel (a, b) and shift (j, i), flattened over (j, i)
        n_idx = ((a + j) * HW_ + (b + i)).reshape(1, 1, PA, PB, 1, 49)
        _GATHER_IDX = np.ascontiguousarray(n_idx)
    return _GATHER_IDX


def _gather_core(corr_k: np.ndarray) -> np.ndarray:
    """[PR, 128, PQ, NF] -> [49, HB, W] band of the output."""
    n_idx = _gather_indices()
    ck = corr_k.reshape(PR, 2, PA, PB, PQ, NF)
    g = np.take_along_axis(ck, n_idx, axis=5)  # [PR, 2, PA, PB, PQ, 49]
    # out[s, pr*8+a, (2q+d)*8+b] = g[pr, d, a, b, q, s]
    g = g.transpose(5, 0, 2, 4, 1, 3).reshape(49, HB, W)
    return g


def _run(in_maps, trace=False, **kw):
    return run_bass_kernel_spmd(
        _program(), in_maps, core_ids=list(range(NCORES)), trace=trace, **kw
    )


def kernel(x: np.ndarray, y: np.ndarray) -> np.ndarray:
    x = np.asarray(x)
    y = np.asarray(y)
    res = _run(_make_in_maps(x, y)).results
    out = np.empty((1, 49, H, W), np.float32)
    for k in range(NCORES):
        out[0, :, k * HB : (k + 1) * HB, :] = _gather_core(
            np.asarray(res[k]["corr"])
        ).astype(np.float32)
    return out
